# revision 89
# baseline (speedup 1.0000x reference)
"""EngramLayer TRN2 kernel v4: 8-core SPMD via bass/tile. 208.6us HW
(574.3us baseline, 2.75x), rel err 7.4e-05.

Sharding: pure data-parallel over tokens (1024/core + 128 halo). The
embedding table is resharded on host per core into a head-group-packed
compact table (unique 4-head row groups actually referenced by that
core's window, 4 adjacent 64-wide rows per group) so every core gathers
its own window locally -- no AllToAll, no collectives at all.

Key findings vs v2 baseline (each verified on HW):
  - SWDGE indirect gathers cost ~5us PER INSTRUCTION (Q7 emission),
    not ~1us: 72 single-row gathers were ~60% of the baseline. Packing
    4 heads per descriptor (256B fetches) cuts this to 18 instrs.
    Multi-column idx APs and dma_gather are BROKEN on this HW (garbage
    descriptors / Q7 crash) -- only [128,1] idx indirect works.
  - one [128tok,256elem] gather block = 2 PE transposes straight into
    feature-major et2b[:, kc, tb] (bf16 PSUM transposes work on HW).
  - h2 (sum hidden^2) computed on HOST, shipped as a tiny input.
  - residual + gate*value epilogue on HOST in f32: kernel ships
    silu(conv) feature-major, value feature-major, and gate.
  - diag(conv_w*scw) tap matrices prebuilt on HOST, DMA'd late.
  - stats_v via PE Gram matmuls (vT^T vT accumulated over hb) +
    DVE identity-mask diag extraction.
  - keys token-major per (tb, m) in PSUM, kc-outer for lhsT reuse;
    k2 via ACT Square(accum), dot via DVE stt(accum) vs bf16 hidden
    (norm weights pre-folded on host; one program for both modes).
  - gate math batched per activation FUNCTION across all m: no ACT
    table set holds both sqrt and sigmoid, so per-m interleaving pays
    ~1.3us per table swap; batching needs only ~4 swaps total.
  - conv taps = 4 accumulating PE matmuls with diagW (tap-outer for
    lhsT reuse), 1-in-4 blocks on a DVE mult-add chain instead;
    AF.Silu works on this HW (v2's "broken" note wrong; CoreSim just
    lacks the table -- sim runs use_silu=False).
  - PSUM triple-buffered in keys/conv phases (bank budget fits after
    scoping pools per phase). Tried and REJECTED on HW measurement:
    fusing keys into the gather loop (234us -- PSUM starvation), vT
    chunks interleaved into the gather loop (model-negative), psc
    bufs=4 (8/8 banks, model-negative). Pool OPEN ORDER matters:
    opening stat/scr before phase 1 costs ~6.5us (model). Moving
    stats_v behind the keys phase: model -5us but HW +7us (215.6us)
    -- the cost model is unreliable for gate-adjacent scheduling
    (no ACT-table-swap modeling); HW-verify anything touching it.
"""

import numpy as np

import concourse.bass as bass
import concourse.bacc as bacc
import concourse.mybir as mybir
import concourse.tile as tile
from concourse.bass_utils import run_bass_kernel_spmd
from concourse.masks import make_identity

F32 = mybir.dt.float32
BF16 = mybir.dt.bfloat16
I32 = mybir.dt.int32
AF = mybir.ActivationFunctionType
ALU = mybir.AluOpType

GATE_EPS = float(np.finfo(np.float32).eps)
SC_EPS = 1e-5


class Cfg:
    def __init__(self, n_cores=8, T_own=1024, VPAD=16384, loop_n=1,
                 use_silu=True, ght_pool=False, pack=4, skip_gather=False,
                 phase_limit=9, no_s1=False, no_s2=False, no_hid=False,
                 debug=False):
        self.n_cores = n_cores
        self.T_own = T_own
        self.VPAD = VPAD
        self.loop_n = loop_n
        self.use_silu = use_silu    # ACT Silu vs Sigmoid+DVE mult
        self.ght_pool = ght_pool    # odd-hb ghT on Pool engine
        self.pack = pack            # heads packed per gather descriptor
        self.skip_gather = skip_gather  # DIAGNOSTIC: plain-DMA et2b
        self.phase_limit = phase_limit  # DIAGNOSTIC: emit phases <= N
        self.no_s1, self.no_s2, self.no_hid = no_s1, no_s2, no_hid
        self.debug = debug
        self.D = 64
        self.HID = 1024
        self.HC = 4
        self.KTAP = 4
        self.DIL = 3
        self.HALO = 128
        self.NH = 8                 # heads
        self.ENG = 512
        self.TP = T_own + self.HALO
        self.NTB = self.TP // 128
        self.C = self.HC * self.HID
        self.KC = self.ENG // 128   # 4
        self.NCT = self.C // 128    # 32 channel blocks
        self.NHB = self.HID // 128  # 8
        self.NM = self.HC * self.NTB
        assert self.T_own % 128 == 0


def _mm_chunks(total, step=512):
    out, c0 = [], 0
    while c0 < total:
        out.append((c0, min(step, total - c0)))
        c0 += step
    return out


def build_program(cfg: Cfg):
    nc = bacc.Bacc("TRN2", target_bir_lowering=False, debug=False,
                   num_devices=cfg.n_cores)

    NGRP = cfg.NH // cfg.pack
    t_tab = nc.dram_tensor("tab", [cfg.VPAD, cfg.D], BF16, kind="ExternalInput")
    t_gidx = nc.dram_tensor("gidx", [128, cfg.NTB * NGRP], I32,
                            kind="ExternalInput")
    t_et2b = (nc.dram_tensor("pre_et2b", [128, cfg.KC, cfg.TP], BF16,
                             kind="ExternalInput") if cfg.skip_gather else None)
    t_hid = nc.dram_tensor("hid", [cfg.TP, cfg.C], BF16, kind="ExternalInput")
    t_h2 = nc.dram_tensor("h2", [128, cfg.NM], F32, kind="ExternalInput")
    t_mask = nc.dram_tensor("mask", [128, 1], F32, kind="ExternalInput")
    t_wvT = nc.dram_tensor("wvt", [128, cfg.KC, cfg.HID], BF16,
                           kind="ExternalInput")
    t_wkT = nc.dram_tensor("wkt", [128, cfg.KC, cfg.C], BF16,
                           kind="ExternalInput")
    t_dgw = nc.dram_tensor("dgw", [128, cfg.NCT, cfg.KTAP, 128], BF16,
                           kind="ExternalInput")
    t_cw = nc.dram_tensor("cw", [128, cfg.NCT, cfg.KTAP], F32,
                          kind="ExternalInput")
    t_out = nc.dram_tensor("out", [cfg.C, cfg.T_own], BF16,
                           kind="ExternalOutput")
    t_vout = nc.dram_tensor("vout", [cfg.HID, cfg.T_own], BF16,
                            kind="ExternalOutput")
    t_gout = nc.dram_tensor("gout", [128, cfg.NM], F32, kind="ExternalOutput")
    t_dbg = None
    if cfg.debug:
        t_dbg = {
            "d_et2b": nc.dram_tensor("d_et2b", [128, cfg.KC, cfg.TP], BF16,
                                     kind="ExternalOutput"),
            "d_vt": nc.dram_tensor("d_vt", [128, cfg.NHB, cfg.TP], BF16,
                                   kind="ExternalOutput"),
        }

    with tile.TileContext(nc) as tc:
        _emit(tc, cfg, t_tab, t_gidx, t_hid, t_h2, t_mask, t_wvT, t_wkT,
              t_dgw, t_cw, t_out, t_vout, t_gout, t_dbg, t_et2b)

    nc.compile()
    return nc


def _emit(tc, cfg, t_tab, t_gidx, t_hid, t_h2, t_mask, t_wvT, t_wkT,
          t_dgw, t_cw, t_out, t_vout, t_gout, t_dbg=None, t_et2b=None):
    nc = tc.nc
    HID, HC, C, KC, TP, NTB, T = (cfg.HID, cfg.HC, cfg.C, cfg.KC, cfg.TP,
                                  cfg.NTB, cfg.T_own)
    NHB, KTAP, DIL, HALO, NH, NM = (cfg.NHB, cfg.KTAP, cfg.DIL, cfg.HALO,
                                    cfg.NH, cfg.NM)

    import contextlib
    loop_cm = None
    if cfg.loop_n > 1:
        loop_cm = tc.For_i(0, cfg.loop_n, 1)
        loop_cm.__enter__()
    ctx = contextlib.ExitStack()
    with ctx:
        const = ctx.enter_context(tc.tile_pool(name="const", bufs=1))
        wts = ctx.enter_context(tc.tile_pool(name="wts", bufs=1))

        # ---------------- constants / weights ----------------
        ident_f = const.tile([128, 128], F32)
        make_identity(nc, ident_f[:])
        ident_b = const.tile([128, 128], BF16)
        make_identity(nc, ident_b[:])
        mask_sb = const.tile([128, 1], F32)
        nc.sync.dma_start(out=mask_sb[:], in_=t_mask[:])
        eps_g = const.tile([128, 1], F32)
        nc.gpsimd.memset(eps_g[:], GATE_EPS)
        eps_s = const.tile([128, 1], F32)
        nc.gpsimd.memset(eps_s[:], SC_EPS)

        NGRP = NH // cfg.pack   # gather groups per token
        KPG = cfg.pack // 2     # kc blocks per gather
        idx_all = wts.tile([128, NTB * NGRP], I32)
        nc.sync.dma_start(out=idx_all[:], in_=t_gidx[:])
        h2s = wts.tile([128, NM], F32)
        nc.sync.dma_start(out=h2s[:], in_=t_h2[:])
        wvT = wts.tile([128, KC, HID], BF16)
        nc.sync.dma_start(out=wvT[:], in_=t_wvT[:])
        wkT = wts.tile([128, KC, C], BF16)
        nc.sync.dma_start(out=wkT[:], in_=t_wkT[:])
        diagW = wts.tile([128, cfg.NCT, KTAP, 128], BF16)

        # ---------------- phase 1: packed-head gather -> et2b ------
        # tab rows hold head groups at adjacent 64-wide rows; one descriptor
        # fetches pack*64 contiguous elems = pack heads for one token.
        # transpose [128tok, 128elem] -> et2b[:, kc, tbslice] directly.
        et2b = wts.tile([128, KC, TP], BF16)
        if t_et2b is not None:
            nc.sync.dma_start(out=et2b[:], in_=t_et2b[:])

        with tc.tile_pool(name="ph1", bufs=4) as ph1, \
             tc.tile_pool(name="ptr", bufs=3, space="PSUM") as ptr:
            for tb in range(NTB if t_et2b is None else 0):
                for gr in range(NGRP):
                    g = tb * NGRP + gr
                    gt = ph1.tile([128, KPG * 128], BF16, tag="gt")
                    nc.gpsimd.indirect_dma_start(
                        out=gt[:], out_offset=None, in_=t_tab[:],
                        in_offset=bass.IndirectOffsetOnAxis(
                            ap=idx_all[:, g:g + 1], axis=0))
                    for i in range(KPG):
                        kc = gr * KPG + i
                        ps = ptr.tile([128, 128], BF16, tag="tr")
                        nc.tensor.transpose(out=ps[:],
                                            in_=gt[:, i * 128:(i + 1) * 128],
                                            identity=ident_b[:])
                        nc.vector.tensor_copy(
                            out=et2b[:, kc, tb * 128:(tb + 1) * 128],
                            in_=ps[:])

        # ---------------- phase 2: vT + stats_v ----------------
        stat = ctx.enter_context(tc.tile_pool(name="stat", bufs=1))
        scr = ctx.enter_context(tc.tile_pool(name="scr", bufs=2))
        vT = wts.tile([128, NHB, TP], BF16)
        with tc.tile_pool(name="pv", bufs=3, space="PSUM") as pv:
            for hb in range(NHB):
                for ci, (c0, cn) in enumerate(_mm_chunks(TP)):
                    ps = pv.tile([128, 512], F32, tag="vmm")
                    for kc in range(KC):
                        nc.tensor.matmul(
                            out=ps[:, :cn],
                            lhsT=wvT[:, kc, hb * 128:(hb + 1) * 128],
                            rhs=et2b[:, kc, c0:c0 + cn],
                            start=(kc == 0), stop=(kc == KC - 1))
                    if (hb + ci) % 2 == 0:   # balance copies ACT/DVE
                        nc.scalar.activation(out=vT[:, hb, c0:c0 + cn],
                                             in_=ps[:, :cn], func=AF.Copy)
                    else:
                        nc.vector.tensor_copy(out=vT[:, hb, c0:c0 + cn],
                                              in_=ps[:, :cn])
            # ship value (own window) while later phases run
            for hb in range(NHB):
                nc.sync.dma_start(
                    out=t_vout[hb * 128:(hb + 1) * 128, :],
                    in_=vT[:, hb, HALO:])

        if t_dbg is not None:
            nc.sync.dma_start(out=t_dbg["d_et2b"][:], in_=et2b[:])
            nc.sync.dma_start(out=t_dbg["d_vt"][:], in_=vT[:])

        # stats_v[tok_p, tb] = sum_hid vT^2 via Gram matmul diag
        stats_v = stat.tile([128, NTB], F32)
        with tc.tile_pool(name="pg", bufs=2, space="PSUM") as pg:
            for tb in range(NTB):
                psD = pg.tile([128, 128], F32, tag="gram")
                for hb in range(NHB):
                    blk = vT[:, hb, tb * 128:(tb + 1) * 128]
                    nc.tensor.matmul(out=psD[:], lhsT=blk, rhs=blk,
                                     start=(hb == 0), stop=(hb == NHB - 1))
                junk = scr.tile([128, 128], F32, tag="dscr")
                nc.vector.scalar_tensor_tensor(
                    out=junk[:], in0=psD[:], scalar=1.0, in1=ident_f[:],
                    op0=ALU.mult, op1=ALU.mult,
                    accum_out=stats_v[:, tb:tb + 1])

        if cfg.phase_limit < 3:
            return
        # ---------------- phase 3: keys + stats (tb outer) ----------------
        k2s = dts = None
        if not cfg.no_s1:
            k2s = stat.tile([128, HC, NTB], F32, name="k2s")
        if not cfg.no_s2:
            dts = stat.tile([128, HC, NTB], F32, name="dts")
        hidp = ctx.enter_context(tc.tile_pool(name="hidp", bufs=4))
        psk_cm = tc.tile_pool(name="psk", bufs=3, space="PSUM")
        psk = psk_cm.__enter__()

        for tb in range(NTB):
            hid_t = hidp.tile([128, C], BF16, tag="hid")
            if not cfg.no_hid:
                nc.sync.dma_start(out=hid_t[:],
                                  in_=t_hid[tb * 128:(tb + 1) * 128, :])
            for m in range(HC):
                kps = psk.tile([128, 1024], F32, tag="kmm")
                for kc in range(KC):       # kc-outer: lhsT reused across nh2
                    for nh2 in range(2):
                        nc.tensor.matmul(
                            out=kps[:, nh2 * 512:(nh2 + 1) * 512],
                            lhsT=et2b[:, kc, tb * 128:(tb + 1) * 128],
                            rhs=wkT[:, kc,
                                    m * HID + nh2 * 512:m * HID + (nh2 + 1) * 512],
                            start=(kc == 0), stop=(kc == KC - 1))
                if not cfg.no_s1:
                    s1 = scr.tile([128, HID], BF16, tag="s1")
                    nc.scalar.activation(out=s1[:], in_=kps[:], func=AF.Square,
                                         accum_out=k2s[:, m, tb:tb + 1])
                if not cfg.no_s2:
                    s2 = scr.tile([128, HID], BF16, tag="s2")
                    nc.vector.scalar_tensor_tensor(
                        out=s2[:], in0=kps[:], scalar=1.0,
                        in1=hid_t[:, m * HID:(m + 1) * HID],
                        op0=ALU.mult, op1=ALU.mult,
                        accum_out=dts[:, m, tb:tb + 1])

        psk_cm.__exit__(None, None, None)
        if cfg.phase_limit < 4:
            dump = (k2s if not cfg.no_s1 else
                    (dts if not cfg.no_s2 else None))
            if dump is not None:
                nc.sync.dma_start(out=t_gout[:],
                                  in_=dump[:].rearrange("p a b -> p (a b)"))
            else:
                nc.sync.dma_start(out=t_gout[:, :NTB], in_=stats_v[:])
            return
        nc.sync.dma_start(out=diagW[:], in_=t_dgw[:])
        cw_sb = wts.tile([128, cfg.NCT, KTAP], F32)
        nc.sync.dma_start(out=cw_sb[:], in_=t_cw[:])

        # ---------------- phase 4: gate math + broadcast (per m) -------
        gp = ctx.enter_context(tc.tile_pool(name="gate", bufs=1))
        repp = ctx.enter_context(tc.tile_pool(name="repp", bufs=4))
        gate_all = gp.tile([128, NM], F32)
        g2rep_m = []
        sqh = float(np.sqrt(HID))
        k2f = k2s[:].rearrange("p a b -> p (a b)")
        dtf = dts[:].rearrange("p a b -> p (a b)")
        with tc.tile_pool(name="pbc", bufs=2, space="PSUM") as pbc, \
             tc.tile_pool(name="gsc", bufs=1) as gsc:
            # batched by activation function to minimize ACT table swaps
            rsq = gsc.tile([128, NM], F32)
            nc.scalar.activation(out=rsq[:], in_=h2s[:], func=AF.Sqrt,
                                 scale=1.0 / HID, bias=eps_g[:])
            nc.vector.reciprocal(out=rsq[:], in_=rsq[:])
            rsk = gsc.tile([128, NM], F32)
            nc.scalar.activation(out=rsk[:], in_=k2f, func=AF.Sqrt,
                                 scale=1.0 / HID, bias=eps_g[:])
            nc.vector.reciprocal(out=rsk[:], in_=rsk[:])
            t2 = gsc.tile([128, NM], F32)
            nc.vector.tensor_tensor(out=t2[:], in0=dtf, in1=rsq[:],
                                    op=ALU.mult)
            nc.vector.tensor_tensor(out=t2[:], in0=t2[:], in1=rsk[:],
                                    op=ALU.mult)
            # q = t2 * rsqrt(max(|t2|, eps*sqrt(H)) * sqrt(H))
            aa = gsc.tile([128, NM], F32)
            nc.vector.tensor_scalar(out=aa[:], in0=t2[:], scalar1=-1.0,
                                    scalar2=None, op0=ALU.mult)
            nc.vector.tensor_tensor(out=aa[:], in0=aa[:], in1=t2[:],
                                    op=ALU.max)
            nc.vector.tensor_scalar_max(out=aa[:], in0=aa[:],
                                        scalar1=sqh * 1e-6)
            rr = gsc.tile([128, NM], F32)
            nc.scalar.activation(out=rr[:], in_=aa[:], func=AF.Sqrt,
                                 scale=sqh)
            nc.vector.reciprocal(out=rr[:], in_=rr[:])
            qq = gsc.tile([128, NM], F32)
            nc.vector.tensor_tensor(out=qq[:], in0=rr[:], in1=t2[:],
                                    op=ALU.mult)
            nc.scalar.activation(out=gate_all[:], in_=qq[:], func=AF.Sigmoid)
            gsq = gsc.tile([128, NM], F32)
            nc.scalar.activation(out=gsq[:], in_=gate_all[:], func=AF.Square)
            for m in range(HC):
                nc.vector.tensor_tensor(out=gsq[:, m * NTB:(m + 1) * NTB],
                                        in0=gsq[:, m * NTB:(m + 1) * NTB],
                                        in1=stats_v[:], op=ALU.mult)
            r2 = gsc.tile([128, NM], F32)
            nc.scalar.activation(out=r2[:], in_=gsq[:], func=AF.Sqrt,
                                 scale=1.0 / HID, bias=eps_s[:])
            nc.vector.reciprocal(out=r2[:], in_=r2[:])
            g2a = gsc.tile([128, NM], F32)
            nc.vector.tensor_tensor(out=g2a[:], in0=gate_all[:], in1=r2[:],
                                    op=ALU.mult)
            for m in range(HC):  # mask halo block for sequence-start cores
                nc.vector.tensor_tensor(out=g2a[:, m * NTB:m * NTB + 1],
                                        in0=g2a[:, m * NTB:m * NTB + 1],
                                        in1=mask_sb[:], op=ALU.mult)
            nc.sync.dma_start(out=t_gout[:], in_=gate_all[:])

            # broadcast per-token g2 to all partitions via PE transpose
            for m in range(HC):
                g2rep = repp.tile([128, TP], BF16, tag="g2rep")
                g2rep_m.append(g2rep)
                for q in range((NTB + 3) // 4):
                    cnt = min(4, NTB - q * 4)
                    psg = pbc.tile([128, 512], F32, tag="brd")
                    for i in range(cnt):
                        tb = q * 4 + i
                        col = m * NTB + tb
                        nc.tensor.transpose(
                            out=psg[:, i * 128:(i + 1) * 128],
                            in_=g2a[:, col:col + 1].to_broadcast([128, 128]),
                            identity=ident_f[:])
                    nc.scalar.activation(
                        out=g2rep[:, q * 512:q * 512 + cnt * 128],
                        in_=psg[:, :cnt * 128], func=AF.Copy)

        if cfg.phase_limit < 5:
            return
        # ---------------- phase 5: conv pipeline per (m, hb) ----------
        off0 = HALO - DIL * (KTAP - 1)  # first tap column offset
        with tc.tile_pool(name="ph3", bufs=4) as ph3, \
             tc.tile_pool(name="psc", bufs=3, space="PSUM") as psc:
            for m in range(HC):
                for hb in range(NHB):
                    ct = m * NHB + hb
                    ghT = ph3.tile([128, TP], BF16, tag="ghT")
                    eng = (nc.gpsimd if (cfg.ght_pool and hb % 2 == 1)
                           else nc.vector)
                    eng.tensor_tensor(out=ghT[:], in0=vT[:, hb],
                                      in1=g2rep_m[m][:], op=ALU.mult)
                    on_dve = hb % 4 == 3   # offload 1/4 of taps to DVE
                    if on_dve:
                        acc = ph3.tile([128, T], BF16, tag="dacc")
                        nc.vector.tensor_scalar(
                            out=acc[:], in0=ghT[:, off0:off0 + T],
                            scalar1=cw_sb[:, ct, 0:1], scalar2=None,
                            op0=ALU.mult)
                        for j in range(1, KTAP):
                            off = off0 + DIL * j
                            nc.vector.scalar_tensor_tensor(
                                out=acc[:], in0=ghT[:, off:off + T],
                                scalar=cw_sb[:, ct, j:j + 1], in1=acc[:],
                                op0=ALU.mult, op1=ALU.add)
                        conv_src = acc[:]
                    else:
                        cps = psc.tile([128, 1024], F32, tag="cmm")
                        for j in range(KTAP):   # tap-outer: one LDW per tap
                            for (c0, cn) in _mm_chunks(T):
                                off = off0 + DIL * j + c0
                                nc.tensor.matmul(out=cps[:, c0:c0 + cn],
                                                 lhsT=diagW[:, ct, j],
                                                 rhs=ghT[:, off:off + cn],
                                                 start=(j == 0),
                                                 stop=(j == KTAP - 1))
                        conv_src = cps[:, :T]
                    silF = ph3.tile([128, T], BF16, tag="silF")
                    if cfg.use_silu:
                        nc.scalar.activation(out=silF[:], in_=conv_src,
                                             func=AF.Silu)
                    else:
                        sig = ph3.tile([128, T], BF16, tag="sig")
                        nc.scalar.activation(out=sig[:], in_=conv_src,
                                             func=AF.Sigmoid)
                        nc.vector.scalar_tensor_tensor(
                            out=silF[:], in0=conv_src, scalar=1.0,
                            in1=sig[:], op0=ALU.mult, op1=ALU.mult)
                    nc.sync.dma_start(
                        out=t_out[ct * 128:(ct + 1) * 128, :], in_=silF[:])
    if loop_cm is not None:
        loop_cm.__exit__(None, None, None)


# ---------------- host side ----------------

def _bf16(a):
    import ml_dtypes
    return np.ascontiguousarray(
        np.asarray(a, np.float32).astype(ml_dtypes.bfloat16))


def make_in_maps(cfg: Cfg, hash_indices, hidden, emb_table, offsets, w_v, w_k,
                 norm_h_w, norm_k_w, sc_norm_w, conv_w, seq_len):
    """hash_indices [NTOK, NH] int32, hidden [NTOK, C] f32."""
    n = cfg.n_cores
    NTOK = hash_indices.shape[0]
    assert NTOK == n * cfg.T_own
    NH, NTB, HALO, HC, HID = cfg.NH, cfg.NTB, cfg.HALO, cfg.HC, cfg.HID

    # weights, host-pretransposed into [128e, kc, ...] bf16
    wv = np.asarray(w_v, np.float32)                      # [HID, ENG]
    wvT = _bf16(wv.T.reshape(cfg.KC, 128, cfg.HID).transpose(1, 0, 2))
    wk = np.asarray(w_k, np.float32).reshape(cfg.C, cfg.ENG)
    wkT = _bf16(wk.T.reshape(cfg.KC, 128, cfg.C).transpose(1, 0, 2))
    cw = (np.asarray(conv_w, np.float32)
          * np.asarray(sc_norm_w, np.float32)[:, :, None]).reshape(
              cfg.C, cfg.KTAP)
    # diagW[p, ct, j, q] = (p==q) * cw[ct*128+p, j]
    dgw = np.zeros((128, cfg.NCT, cfg.KTAP, 128), np.float32)
    cwr = cw.reshape(cfg.NCT, 128, cfg.KTAP)
    pr = np.arange(128)
    dgw[pr, :, :, pr] = cwr.transpose(1, 0, 2)
    dgw = _bf16(dgw)
    cw_in = np.ascontiguousarray(cwr.transpose(1, 0, 2), np.float32)

    wpr = (np.asarray(norm_h_w, np.float32)
           * np.asarray(norm_k_w, np.float32)).reshape(cfg.C)
    hid_f = np.asarray(hidden, np.float32)
    hid_eff = _bf16(hid_f * wpr[None, :]) if not np.allclose(wpr, 1.0) \
        else _bf16(hid_f)
    # h2 per token from raw hidden (f32, exact)
    h2_tok = (hid_f.reshape(NTOK, HC, HID) ** 2).sum(axis=2)  # [NTOK, HC]

    glob_rows = (np.asarray(hash_indices, np.int64)
                 + np.asarray(offsets, np.int64)[None, :])   # [NTOK, NH]

    import ml_dtypes
    emb_f = np.asarray(emb_table, np.float32)
    TP, KC = cfg.TP, cfg.KC
    in_maps = []
    for c in range(n):
        hwin = np.clip(np.arange(c * cfg.T_own - HALO,
                                 c * cfg.T_own + cfg.T_own), 0, NTOK - 1)
        rows = glob_rows[hwin]                       # [TP, NH]
        # head-group packing: group key = tuple of `pack` row ids; each
        # group becomes `pack` adjacent 64-wide table rows -> one fetch.
        P = cfg.pack
        NGRP = NH // P
        grp = np.ascontiguousarray(rows.reshape(TP * NGRP, P))
        uk, inv = np.unique(grp, axis=0, return_inverse=True)
        inv = inv.reshape(TP, NGRP)
        n_grp = len(uk)
        assert P * n_grp <= cfg.VPAD
        tab = np.zeros((cfg.VPAD, cfg.D), dtype=ml_dtypes.bfloat16)
        for i in range(P):
            tab[i:P * n_grp:P] = _bf16(emb_f[uk[:, i]])
        # gidx[p, tb*NGRP + gr] = P*group_index(token tb*128+p, gr)
        gidx = (P * inv.reshape(NTB, 128, NGRP).transpose(1, 0, 2)
                .reshape(128, NTB * NGRP)).astype(np.int32)
        gidx = np.ascontiguousarray(gidx)

        h2w = h2_tok[hwin].reshape(NTB, 128, HC).transpose(1, 2, 0)
        h2w = np.ascontiguousarray(h2w.reshape(128, HC * NTB), np.float32)

        halo_valid = (c * cfg.T_own) % seq_len != 0
        mask = np.full((128, 1), 1.0 if halo_valid else 0.0, np.float32)

        im = {
            "tab": tab,
            "gidx": gidx,
            "hid": np.ascontiguousarray(hid_eff[hwin]),
            "h2": h2w,
            "mask": mask,
            "wvt": wvT, "wkt": wkT, "dgw": dgw, "cw": cw_in,
        }
        if cfg.skip_gather:
            embw = _bf16(emb_f[rows]).reshape(TP, cfg.NH * cfg.D)
            pre = np.ascontiguousarray(
                embw.T.reshape(cfg.KC, 128, TP).transpose(1, 0, 2))
            im["pre_et2b"] = pre
        in_maps.append(im)
    return in_maps


def assemble_output(cfg: Cfg, results, hidden_f32):
    """results: list of per-core dicts with out/vout/gout.
    hidden_f32: [NTOK, HC, HID]. Returns [NTOK, HC, HID] f32."""
    n, T, HC, HID, NTB = cfg.n_cores, cfg.T_own, cfg.HC, cfg.HID, cfg.NTB
    outs = []
    for c in range(n):
        sil = np.asarray(results[c]["out"], np.float32)      # [C, T]
        sil = sil.reshape(HC, HID, T).transpose(2, 0, 1)     # [T, HC, HID]
        val = np.asarray(results[c]["vout"], np.float32).T   # [T, HID]
        g = np.asarray(results[c]["gout"], np.float32)       # [128, HC*NTB]
        g = g.reshape(128, HC, NTB)[:, :, 1:]                # drop halo block
        gate = g.transpose(2, 0, 1).reshape(T, HC)           # [T, HC]
        out = sil + gate[:, :, None] * val[:, None, :]
        outs.append(out)
    full = np.concatenate(outs, axis=0)
    return full + hidden_f32


_CACHED = {}


def _get_program(key, cfg):
    if key not in _CACHED:
        _CACHED[key] = build_program(cfg)
    return _CACHED[key]


def prepare(inputs, cfg=None):
    hash_indices = np.asarray(inputs["hash_indices"])
    hidden = np.asarray(inputs["hidden_states"], dtype=np.float32)
    B, S, NH = hash_indices.shape
    NTOK = B * S
    if cfg is None:
        cfg = Cfg(n_cores=8, T_own=NTOK // 8)
    nc = _get_program(("hw", cfg.T_own, cfg.VPAD, cfg.use_silu,
                       cfg.ght_pool, cfg.loop_n, cfg.pack,
                       cfg.skip_gather, cfg.phase_limit), cfg)
    hidx = np.ascontiguousarray(
        hash_indices.reshape(NTOK, NH).astype(np.int32))
    hid_flat = np.ascontiguousarray(hidden.reshape(NTOK, cfg.C))
    in_maps = make_in_maps(
        cfg, hidx, hid_flat,
        np.asarray(inputs["emb_table"], np.float32),
        np.asarray(inputs["offsets"]),
        np.asarray(inputs["w_v"], np.float32),
        np.asarray(inputs["w_k"], np.float32),
        np.asarray(inputs["norm_h_w"], np.float32),
        np.asarray(inputs["norm_k_w"], np.float32),
        np.asarray(inputs["sc_norm_w"], np.float32),
        np.asarray(inputs["conv_w"], np.float32),
        seq_len=S)
    return cfg, nc, in_maps


def kernel(**inputs):
    hidden = np.asarray(inputs["hidden_states"], dtype=np.float32)
    B, S, NH = np.asarray(inputs["hash_indices"]).shape
    NTOK = B * S
    cfg, nc, in_maps = prepare(inputs)
    res = run_bass_kernel_spmd(nc, in_maps, core_ids=list(range(8)))
    full = assemble_output(cfg, res.results,
                           hidden.reshape(NTOK, cfg.HC, cfg.HID))
    return full.reshape(B, S, cfg.HC, cfg.HID).astype(np.float32)


# revision 90
# speedup vs baseline: 1.0469x; 1.0469x over previous
"""EngramLayer TRN2 kernel v4: 8-core SPMD via bass/tile. 208.6us HW
(574.3us baseline, 2.75x), rel err 7.4e-05.

Sharding: pure data-parallel over tokens (1024/core + 128 halo). The
embedding table is resharded on host per core into a head-group-packed
compact table (unique 4-head row groups actually referenced by that
core's window, 4 adjacent 64-wide rows per group) so every core gathers
its own window locally -- no AllToAll, no collectives at all.

Key findings vs v2 baseline (each verified on HW):
  - SWDGE indirect gathers cost ~5us PER INSTRUCTION (Q7 emission),
    not ~1us: 72 single-row gathers were ~60% of the baseline. Packing
    4 heads per descriptor (256B fetches) cuts this to 18 instrs.
    Multi-column idx APs and dma_gather are BROKEN on this HW (garbage
    descriptors / Q7 crash) -- only [128,1] idx indirect works.
  - one [128tok,256elem] gather block = 2 PE transposes straight into
    feature-major et2b[:, kc, tb] (bf16 PSUM transposes work on HW).
  - h2 (sum hidden^2) computed on HOST, shipped as a tiny input.
  - residual + gate*value epilogue on HOST in f32: kernel ships
    silu(conv) feature-major, value feature-major, and gate.
  - diag(conv_w*scw) tap matrices prebuilt on HOST, DMA'd late.
  - stats_v via PE Gram matmuls (vT^T vT accumulated over hb) +
    DVE identity-mask diag extraction.
  - keys token-major per (tb, m) in PSUM, kc-outer for lhsT reuse;
    k2 via ACT Square(accum), dot via DVE stt(accum) vs bf16 hidden
    (norm weights pre-folded on host; one program for both modes).
  - gate math batched per activation FUNCTION across all m: no ACT
    table set holds both sqrt and sigmoid, so per-m interleaving pays
    ~1.3us per table swap; batching needs only ~4 swaps total.
  - conv taps = 4 accumulating PE matmuls with diagW (tap-outer for
    lhsT reuse), 1-in-4 blocks on a DVE mult-add chain instead;
    AF.Silu works on this HW (v2's "broken" note wrong; CoreSim just
    lacks the table -- sim runs use_silu=False).
  - PSUM triple-buffered in keys/conv phases (bank budget fits after
    scoping pools per phase). Tried and REJECTED on HW measurement:
    fusing keys into the gather loop (234us -- PSUM starvation), vT
    chunks interleaved into the gather loop (model-negative), psc
    bufs=4 (8/8 banks, model-negative). Pool OPEN ORDER matters:
    opening stat/scr before phase 1 costs ~6.5us (model). Moving
    stats_v behind the keys phase: model -5us but HW +7us (215.6us)
    -- the cost model is unreliable for gate-adjacent scheduling
    (no ACT-table-swap modeling); HW-verify anything touching it.
"""

import numpy as np

import concourse.bass as bass
import concourse.bacc as bacc
import concourse.mybir as mybir
import concourse.tile as tile
from concourse.bass_utils import run_bass_kernel_spmd
from concourse.masks import make_identity

F32 = mybir.dt.float32
BF16 = mybir.dt.bfloat16
I32 = mybir.dt.int32
AF = mybir.ActivationFunctionType
ALU = mybir.AluOpType

GATE_EPS = float(np.finfo(np.float32).eps)
SC_EPS = 1e-5


class Cfg:
    def __init__(self, n_cores=8, T_own=1024, VPAD=16384, loop_n=1,
                 use_silu=True, ght_pool=False, pack=4, skip_gather=False,
                 phase_limit=9, no_s1=False, no_s2=False, no_hid=False,
                 debug=False):
        self.n_cores = n_cores
        self.T_own = T_own
        self.VPAD = VPAD
        self.loop_n = loop_n
        self.use_silu = use_silu    # ACT Silu vs Sigmoid+DVE mult
        self.ght_pool = ght_pool    # odd-hb ghT on Pool engine
        self.pack = pack            # heads packed per gather descriptor
        self.skip_gather = skip_gather  # DIAGNOSTIC: plain-DMA et2b
        self.phase_limit = phase_limit  # DIAGNOSTIC: emit phases <= N
        self.no_s1, self.no_s2, self.no_hid = no_s1, no_s2, no_hid
        self.debug = debug
        self.D = 64
        self.HID = 1024
        self.HC = 4
        self.KTAP = 4
        self.DIL = 3
        self.HALO = 128
        self.NH = 8                 # heads
        self.ENG = 512
        self.TP = T_own + self.HALO
        self.NTB = self.TP // 128
        self.C = self.HC * self.HID
        self.KC = self.ENG // 128   # 4
        self.NCT = self.C // 128    # 32 channel blocks
        self.NHB = self.HID // 128  # 8
        self.NM = self.HC * self.NTB
        assert self.T_own % 128 == 0


def _mm_chunks(total, step=512):
    out, c0 = [], 0
    while c0 < total:
        out.append((c0, min(step, total - c0)))
        c0 += step
    return out


def build_program(cfg: Cfg):
    nc = bacc.Bacc("TRN2", target_bir_lowering=False, debug=False,
                   num_devices=cfg.n_cores)

    NGRP = cfg.NH // cfg.pack
    t_tab = nc.dram_tensor("tab", [cfg.VPAD, cfg.D], BF16, kind="ExternalInput")
    t_gidx = nc.dram_tensor("gidx", [128, cfg.NTB * NGRP], I32,
                            kind="ExternalInput")
    t_et2b = (nc.dram_tensor("pre_et2b", [128, cfg.KC, cfg.TP], BF16,
                             kind="ExternalInput") if cfg.skip_gather else None)
    t_hid = nc.dram_tensor("hid", [cfg.TP, cfg.C], BF16, kind="ExternalInput")
    t_h2 = nc.dram_tensor("h2", [128, cfg.NM], F32, kind="ExternalInput")
    t_mask = nc.dram_tensor("mask", [128, 1], F32, kind="ExternalInput")
    t_wvT = nc.dram_tensor("wvt", [128, cfg.KC, cfg.HID], BF16,
                           kind="ExternalInput")
    t_wkT = nc.dram_tensor("wkt", [128, cfg.KC, cfg.C], BF16,
                           kind="ExternalInput")
    t_dgw = nc.dram_tensor("dgw", [128, cfg.NCT, cfg.KTAP, 128], BF16,
                           kind="ExternalInput")
    t_cw = nc.dram_tensor("cw", [128, cfg.NCT, cfg.KTAP], F32,
                          kind="ExternalInput")
    t_out = nc.dram_tensor("out", [cfg.C, cfg.T_own], BF16,
                           kind="ExternalOutput")
    t_vout = nc.dram_tensor("vout", [cfg.HID, cfg.T_own], BF16,
                            kind="ExternalOutput")
    t_gout = nc.dram_tensor("gout", [128, cfg.NM], F32, kind="ExternalOutput")
    t_dbg = None
    if cfg.debug:
        t_dbg = {
            "d_et2b": nc.dram_tensor("d_et2b", [128, cfg.KC, cfg.TP], BF16,
                                     kind="ExternalOutput"),
            "d_vt": nc.dram_tensor("d_vt", [128, cfg.NHB, cfg.TP], BF16,
                                   kind="ExternalOutput"),
        }

    with tile.TileContext(nc) as tc:
        _emit(tc, cfg, t_tab, t_gidx, t_hid, t_h2, t_mask, t_wvT, t_wkT,
              t_dgw, t_cw, t_out, t_vout, t_gout, t_dbg, t_et2b)

    nc.compile()
    return nc


def _emit(tc, cfg, t_tab, t_gidx, t_hid, t_h2, t_mask, t_wvT, t_wkT,
          t_dgw, t_cw, t_out, t_vout, t_gout, t_dbg=None, t_et2b=None):
    nc = tc.nc
    HID, HC, C, KC, TP, NTB, T = (cfg.HID, cfg.HC, cfg.C, cfg.KC, cfg.TP,
                                  cfg.NTB, cfg.T_own)
    NHB, KTAP, DIL, HALO, NH, NM = (cfg.NHB, cfg.KTAP, cfg.DIL, cfg.HALO,
                                    cfg.NH, cfg.NM)

    import contextlib
    loop_cm = None
    if cfg.loop_n > 1:
        loop_cm = tc.For_i(0, cfg.loop_n, 1)
        loop_cm.__enter__()
    ctx = contextlib.ExitStack()
    with ctx:
        const = ctx.enter_context(tc.tile_pool(name="const", bufs=1))
        wts = ctx.enter_context(tc.tile_pool(name="wts", bufs=1))

        # ---------------- constants / weights ----------------
        ident_f = const.tile([128, 128], F32)
        make_identity(nc, ident_f[:])
        ident_b = const.tile([128, 128], BF16)
        make_identity(nc, ident_b[:])
        mask_sb = const.tile([128, 1], F32)
        nc.sync.dma_start(out=mask_sb[:], in_=t_mask[:])
        eps_g = const.tile([128, 1], F32)
        nc.gpsimd.memset(eps_g[:], GATE_EPS)
        eps_s = const.tile([128, 1], F32)
        nc.gpsimd.memset(eps_s[:], SC_EPS)

        NGRP = NH // cfg.pack   # gather groups per token
        KPG = cfg.pack // 2     # kc blocks per gather
        idx_all = wts.tile([128, NTB * NGRP], I32)
        nc.sync.dma_start(out=idx_all[:], in_=t_gidx[:])
        h2s = wts.tile([128, NM], F32)
        nc.sync.dma_start(out=h2s[:], in_=t_h2[:])
        wvT = wts.tile([128, KC, HID], BF16)
        nc.sync.dma_start(out=wvT[:], in_=t_wvT[:])
        wkT = wts.tile([128, KC, C], BF16)
        nc.sync.dma_start(out=wkT[:], in_=t_wkT[:])
        diagW = wts.tile([128, cfg.NCT, KTAP, 128], BF16)

        # ---------------- phase 1: packed-head gather -> et2b ------
        # tab rows hold head groups at adjacent 64-wide rows; one descriptor
        # fetches pack*64 contiguous elems = pack heads for one token.
        # transpose [128tok, 128elem] -> et2b[:, kc, tbslice] directly.
        et2b = wts.tile([128, KC, TP], BF16)
        if t_et2b is not None:
            nc.sync.dma_start(out=et2b[:], in_=t_et2b[:])

        with tc.tile_pool(name="ph1", bufs=3) as ph1, \
             tc.tile_pool(name="ptr", bufs=3, space="PSUM") as ptr:
            for tb in range(NTB if t_et2b is None else 0):
                for gr in range(NGRP):
                    g = tb * NGRP + gr
                    gt = ph1.tile([128, KPG * 128], BF16, tag="gt")
                    nc.gpsimd.indirect_dma_start(
                        out=gt[:], out_offset=None, in_=t_tab[:],
                        in_offset=bass.IndirectOffsetOnAxis(
                            ap=idx_all[:, g:g + 1], axis=0))
                    for i in range(KPG):
                        kc = gr * KPG + i
                        ps = ptr.tile([128, 128], BF16, tag="tr")
                        nc.tensor.transpose(out=ps[:],
                                            in_=gt[:, i * 128:(i + 1) * 128],
                                            identity=ident_b[:])
                        nc.vector.tensor_copy(
                            out=et2b[:, kc, tb * 128:(tb + 1) * 128],
                            in_=ps[:])

        # ---------------- phase 2: vT + stats_v ----------------
        stat = ctx.enter_context(tc.tile_pool(name="stat", bufs=1))
        scr = ctx.enter_context(tc.tile_pool(name="scr", bufs=2))
        vT = wts.tile([128, NHB, TP], BF16)
        with tc.tile_pool(name="pv", bufs=3, space="PSUM") as pv:
            for hb in range(NHB):
                for ci, (c0, cn) in enumerate(_mm_chunks(TP)):
                    ps = pv.tile([128, 512], F32, tag="vmm")
                    for kc in range(KC):
                        nc.tensor.matmul(
                            out=ps[:, :cn],
                            lhsT=wvT[:, kc, hb * 128:(hb + 1) * 128],
                            rhs=et2b[:, kc, c0:c0 + cn],
                            start=(kc == 0), stop=(kc == KC - 1))
                    if (hb + ci) % 2 == 0:   # balance copies ACT/DVE
                        nc.scalar.activation(out=vT[:, hb, c0:c0 + cn],
                                             in_=ps[:, :cn], func=AF.Copy)
                    else:
                        nc.vector.tensor_copy(out=vT[:, hb, c0:c0 + cn],
                                              in_=ps[:, :cn])
            # ship value (own window) while later phases run
            for hb in range(NHB):
                nc.sync.dma_start(
                    out=t_vout[hb * 128:(hb + 1) * 128, :],
                    in_=vT[:, hb, HALO:])

        if t_dbg is not None:
            nc.sync.dma_start(out=t_dbg["d_et2b"][:], in_=et2b[:])
            nc.sync.dma_start(out=t_dbg["d_vt"][:], in_=vT[:])

        # stats_v[tok_p, tb] = sum_hid vT^2 via Gram matmul diag
        stats_v = stat.tile([128, NTB], F32)
        with tc.tile_pool(name="pg", bufs=2, space="PSUM") as pg:
            for tb in range(NTB):
                psD = pg.tile([128, 128], F32, tag="gram")
                for hb in range(NHB):
                    blk = vT[:, hb, tb * 128:(tb + 1) * 128]
                    nc.tensor.matmul(out=psD[:], lhsT=blk, rhs=blk,
                                     start=(hb == 0), stop=(hb == NHB - 1))
                junk = scr.tile([128, 128], F32, tag="dscr")
                nc.vector.scalar_tensor_tensor(
                    out=junk[:], in0=psD[:], scalar=1.0, in1=ident_f[:],
                    op0=ALU.mult, op1=ALU.mult,
                    accum_out=stats_v[:, tb:tb + 1])

        if cfg.phase_limit < 3:
            return
        # ---------------- phase 3: keys + stats (tb outer) ----------------
        k2s = dts = None
        if not cfg.no_s1:
            k2s = stat.tile([128, HC, NTB], F32, name="k2s")
        if not cfg.no_s2:
            dts = stat.tile([128, HC, NTB], F32, name="dts")
        hidp = ctx.enter_context(tc.tile_pool(name="hidp", bufs=3))
        psk_cm = tc.tile_pool(name="psk", bufs=3, space="PSUM")
        psk = psk_cm.__enter__()

        for tb in range(NTB):
            hid_t = hidp.tile([128, C], BF16, tag="hid")
            if not cfg.no_hid:
                nc.sync.dma_start(out=hid_t[:],
                                  in_=t_hid[tb * 128:(tb + 1) * 128, :])
            for m in range(HC):
                kps = psk.tile([128, 1024], F32, tag="kmm")
                for kc in range(KC):       # kc-outer: lhsT reused across nh2
                    for nh2 in range(2):
                        nc.tensor.matmul(
                            out=kps[:, nh2 * 512:(nh2 + 1) * 512],
                            lhsT=et2b[:, kc, tb * 128:(tb + 1) * 128],
                            rhs=wkT[:, kc,
                                    m * HID + nh2 * 512:m * HID + (nh2 + 1) * 512],
                            start=(kc == 0), stop=(kc == KC - 1))
                if not cfg.no_s1:
                    s1 = scr.tile([128, HID], BF16, tag="s1")
                    nc.scalar.activation(out=s1[:], in_=kps[:], func=AF.Square,
                                         accum_out=k2s[:, m, tb:tb + 1])
                if not cfg.no_s2:
                    s2 = scr.tile([128, HID], BF16, tag="s2")
                    nc.vector.scalar_tensor_tensor(
                        out=s2[:], in0=kps[:], scalar=1.0,
                        in1=hid_t[:, m * HID:(m + 1) * HID],
                        op0=ALU.mult, op1=ALU.mult,
                        accum_out=dts[:, m, tb:tb + 1])

        psk_cm.__exit__(None, None, None)
        if cfg.phase_limit < 4:
            dump = (k2s if not cfg.no_s1 else
                    (dts if not cfg.no_s2 else None))
            if dump is not None:
                nc.sync.dma_start(out=t_gout[:],
                                  in_=dump[:].rearrange("p a b -> p (a b)"))
            else:
                nc.sync.dma_start(out=t_gout[:, :NTB], in_=stats_v[:])
            return
        nc.sync.dma_start(out=diagW[:], in_=t_dgw[:])
        cw_sb = wts.tile([128, cfg.NCT, KTAP], F32)
        nc.sync.dma_start(out=cw_sb[:], in_=t_cw[:])

        # ---------------- phase 4: gate math + broadcast (per m) -------
        gp = ctx.enter_context(tc.tile_pool(name="gate", bufs=1))
        repp = ctx.enter_context(tc.tile_pool(name="repp", bufs=4))
        gate_all = gp.tile([128, NM], F32)
        g2rep_m = []
        sqh = float(np.sqrt(HID))
        k2f = k2s[:].rearrange("p a b -> p (a b)")
        dtf = dts[:].rearrange("p a b -> p (a b)")
        with tc.tile_pool(name="pbc", bufs=2, space="PSUM") as pbc, \
             tc.tile_pool(name="gsc", bufs=1) as gsc:
            # batched by activation function to minimize ACT table swaps
            rsq = gsc.tile([128, NM], F32)
            nc.scalar.activation(out=rsq[:], in_=h2s[:], func=AF.Sqrt,
                                 scale=1.0 / HID, bias=eps_g[:])
            nc.vector.reciprocal(out=rsq[:], in_=rsq[:])
            rsk = gsc.tile([128, NM], F32)
            nc.scalar.activation(out=rsk[:], in_=k2f, func=AF.Sqrt,
                                 scale=1.0 / HID, bias=eps_g[:])
            nc.vector.reciprocal(out=rsk[:], in_=rsk[:])
            t2 = gsc.tile([128, NM], F32)
            nc.vector.tensor_tensor(out=t2[:], in0=dtf, in1=rsq[:],
                                    op=ALU.mult)
            nc.vector.tensor_tensor(out=t2[:], in0=t2[:], in1=rsk[:],
                                    op=ALU.mult)
            # q = t2 * rsqrt(max(|t2|, eps*sqrt(H)) * sqrt(H))
            aa = gsc.tile([128, NM], F32)
            nc.vector.tensor_scalar(out=aa[:], in0=t2[:], scalar1=-1.0,
                                    scalar2=None, op0=ALU.mult)
            nc.vector.tensor_tensor(out=aa[:], in0=aa[:], in1=t2[:],
                                    op=ALU.max)
            nc.vector.tensor_scalar_max(out=aa[:], in0=aa[:],
                                        scalar1=sqh * 1e-6)
            rr = gsc.tile([128, NM], F32)
            nc.scalar.activation(out=rr[:], in_=aa[:], func=AF.Sqrt,
                                 scale=sqh)
            nc.vector.reciprocal(out=rr[:], in_=rr[:])
            qq = gsc.tile([128, NM], F32)
            nc.vector.tensor_tensor(out=qq[:], in0=rr[:], in1=t2[:],
                                    op=ALU.mult)
            nc.scalar.activation(out=gate_all[:], in_=qq[:], func=AF.Sigmoid)
            gsq = gsc.tile([128, NM], F32)
            nc.scalar.activation(out=gsq[:], in_=gate_all[:], func=AF.Square)
            for m in range(HC):
                nc.vector.tensor_tensor(out=gsq[:, m * NTB:(m + 1) * NTB],
                                        in0=gsq[:, m * NTB:(m + 1) * NTB],
                                        in1=stats_v[:], op=ALU.mult)
            r2 = gsc.tile([128, NM], F32)
            nc.scalar.activation(out=r2[:], in_=gsq[:], func=AF.Sqrt,
                                 scale=1.0 / HID, bias=eps_s[:])
            nc.vector.reciprocal(out=r2[:], in_=r2[:])
            g2a = gsc.tile([128, NM], F32)
            nc.vector.tensor_tensor(out=g2a[:], in0=gate_all[:], in1=r2[:],
                                    op=ALU.mult)
            for m in range(HC):  # mask halo block for sequence-start cores
                nc.vector.tensor_tensor(out=g2a[:, m * NTB:m * NTB + 1],
                                        in0=g2a[:, m * NTB:m * NTB + 1],
                                        in1=mask_sb[:], op=ALU.mult)
            nc.sync.dma_start(out=t_gout[:], in_=gate_all[:])

            # broadcast per-token g2 to all partitions via PE transpose
            for m in range(HC):
                g2rep = repp.tile([128, TP], BF16, tag="g2rep")
                g2rep_m.append(g2rep)
                for q in range((NTB + 3) // 4):
                    cnt = min(4, NTB - q * 4)
                    psg = pbc.tile([128, 512], F32, tag="brd")
                    for i in range(cnt):
                        tb = q * 4 + i
                        col = m * NTB + tb
                        nc.tensor.transpose(
                            out=psg[:, i * 128:(i + 1) * 128],
                            in_=g2a[:, col:col + 1].to_broadcast([128, 128]),
                            identity=ident_f[:])
                    nc.scalar.activation(
                        out=g2rep[:, q * 512:q * 512 + cnt * 128],
                        in_=psg[:, :cnt * 128], func=AF.Copy)

        if cfg.phase_limit < 5:
            return
        # ---------------- phase 5: conv pipeline per (m, hb) ----------
        off0 = HALO - DIL * (KTAP - 1)  # first tap column offset
        with tc.tile_pool(name="ph3", bufs=4) as ph3, \
             tc.tile_pool(name="psc", bufs=3, space="PSUM") as psc:
            for m in range(HC):
                for hb in range(NHB):
                    ct = m * NHB + hb
                    ghT = ph3.tile([128, TP], BF16, tag="ghT")
                    eng = (nc.gpsimd if (cfg.ght_pool and hb % 2 == 1)
                           else nc.vector)
                    eng.tensor_tensor(out=ghT[:], in0=vT[:, hb],
                                      in1=g2rep_m[m][:], op=ALU.mult)
                    on_dve = hb % 4 == 3   # offload 1/4 of taps to DVE
                    if on_dve:
                        acc = ph3.tile([128, T], BF16, tag="dacc")
                        nc.vector.tensor_scalar(
                            out=acc[:], in0=ghT[:, off0:off0 + T],
                            scalar1=cw_sb[:, ct, 0:1], scalar2=None,
                            op0=ALU.mult)
                        for j in range(1, KTAP):
                            off = off0 + DIL * j
                            nc.vector.scalar_tensor_tensor(
                                out=acc[:], in0=ghT[:, off:off + T],
                                scalar=cw_sb[:, ct, j:j + 1], in1=acc[:],
                                op0=ALU.mult, op1=ALU.add)
                        conv_src = acc[:]
                    else:
                        cps = psc.tile([128, 1024], F32, tag="cmm")
                        for j in range(KTAP):   # tap-outer: one LDW per tap
                            for (c0, cn) in _mm_chunks(T):
                                off = off0 + DIL * j + c0
                                nc.tensor.matmul(out=cps[:, c0:c0 + cn],
                                                 lhsT=diagW[:, ct, j],
                                                 rhs=ghT[:, off:off + cn],
                                                 start=(j == 0),
                                                 stop=(j == KTAP - 1))
                        conv_src = cps[:, :T]
                    silF = ph3.tile([128, T], BF16, tag="silF")
                    if cfg.use_silu:
                        nc.scalar.activation(out=silF[:], in_=conv_src,
                                             func=AF.Silu)
                    else:
                        sig = ph3.tile([128, T], BF16, tag="sig")
                        nc.scalar.activation(out=sig[:], in_=conv_src,
                                             func=AF.Sigmoid)
                        nc.vector.scalar_tensor_tensor(
                            out=silF[:], in0=conv_src, scalar=1.0,
                            in1=sig[:], op0=ALU.mult, op1=ALU.mult)
                    nc.sync.dma_start(
                        out=t_out[ct * 128:(ct + 1) * 128, :], in_=silF[:])
    if loop_cm is not None:
        loop_cm.__exit__(None, None, None)


# ---------------- host side ----------------

def _bf16(a):
    import ml_dtypes
    return np.ascontiguousarray(
        np.asarray(a, np.float32).astype(ml_dtypes.bfloat16))


def make_in_maps(cfg: Cfg, hash_indices, hidden, emb_table, offsets, w_v, w_k,
                 norm_h_w, norm_k_w, sc_norm_w, conv_w, seq_len):
    """hash_indices [NTOK, NH] int32, hidden [NTOK, C] f32."""
    n = cfg.n_cores
    NTOK = hash_indices.shape[0]
    assert NTOK == n * cfg.T_own
    NH, NTB, HALO, HC, HID = cfg.NH, cfg.NTB, cfg.HALO, cfg.HC, cfg.HID

    # weights, host-pretransposed into [128e, kc, ...] bf16
    wv = np.asarray(w_v, np.float32)                      # [HID, ENG]
    wvT = _bf16(wv.T.reshape(cfg.KC, 128, cfg.HID).transpose(1, 0, 2))
    wk = np.asarray(w_k, np.float32).reshape(cfg.C, cfg.ENG)
    wkT = _bf16(wk.T.reshape(cfg.KC, 128, cfg.C).transpose(1, 0, 2))
    cw = (np.asarray(conv_w, np.float32)
          * np.asarray(sc_norm_w, np.float32)[:, :, None]).reshape(
              cfg.C, cfg.KTAP)
    # diagW[p, ct, j, q] = (p==q) * cw[ct*128+p, j]
    dgw = np.zeros((128, cfg.NCT, cfg.KTAP, 128), np.float32)
    cwr = cw.reshape(cfg.NCT, 128, cfg.KTAP)
    pr = np.arange(128)
    dgw[pr, :, :, pr] = cwr.transpose(1, 0, 2)
    dgw = _bf16(dgw)
    cw_in = np.ascontiguousarray(cwr.transpose(1, 0, 2), np.float32)

    wpr = (np.asarray(norm_h_w, np.float32)
           * np.asarray(norm_k_w, np.float32)).reshape(cfg.C)
    hid_f = np.asarray(hidden, np.float32)
    hid_eff = _bf16(hid_f * wpr[None, :]) if not np.allclose(wpr, 1.0) \
        else _bf16(hid_f)
    # h2 per token from raw hidden (f32, exact)
    h2_tok = (hid_f.reshape(NTOK, HC, HID) ** 2).sum(axis=2)  # [NTOK, HC]

    glob_rows = (np.asarray(hash_indices, np.int64)
                 + np.asarray(offsets, np.int64)[None, :])   # [NTOK, NH]

    import ml_dtypes
    emb_f = np.asarray(emb_table, np.float32)
    TP, KC = cfg.TP, cfg.KC
    in_maps = []
    for c in range(n):
        hwin = np.clip(np.arange(c * cfg.T_own - HALO,
                                 c * cfg.T_own + cfg.T_own), 0, NTOK - 1)
        rows = glob_rows[hwin]                       # [TP, NH]
        # head-group packing: group key = tuple of `pack` row ids; each
        # group becomes `pack` adjacent 64-wide table rows -> one fetch.
        P = cfg.pack
        NGRP = NH // P
        grp = np.ascontiguousarray(rows.reshape(TP * NGRP, P))
        uk, inv = np.unique(grp, axis=0, return_inverse=True)
        inv = inv.reshape(TP, NGRP)
        n_grp = len(uk)
        assert P * n_grp <= cfg.VPAD
        tab = np.zeros((cfg.VPAD, cfg.D), dtype=ml_dtypes.bfloat16)
        for i in range(P):
            tab[i:P * n_grp:P] = _bf16(emb_f[uk[:, i]])
        # gidx[p, tb*NGRP + gr] = P*group_index(token tb*128+p, gr)
        gidx = (P * inv.reshape(NTB, 128, NGRP).transpose(1, 0, 2)
                .reshape(128, NTB * NGRP)).astype(np.int32)
        gidx = np.ascontiguousarray(gidx)

        h2w = h2_tok[hwin].reshape(NTB, 128, HC).transpose(1, 2, 0)
        h2w = np.ascontiguousarray(h2w.reshape(128, HC * NTB), np.float32)

        halo_valid = (c * cfg.T_own) % seq_len != 0
        mask = np.full((128, 1), 1.0 if halo_valid else 0.0, np.float32)

        im = {
            "tab": tab,
            "gidx": gidx,
            "hid": np.ascontiguousarray(hid_eff[hwin]),
            "h2": h2w,
            "mask": mask,
            "wvt": wvT, "wkt": wkT, "dgw": dgw, "cw": cw_in,
        }
        if cfg.skip_gather:
            embw = _bf16(emb_f[rows]).reshape(TP, cfg.NH * cfg.D)
            pre = np.ascontiguousarray(
                embw.T.reshape(cfg.KC, 128, TP).transpose(1, 0, 2))
            im["pre_et2b"] = pre
        in_maps.append(im)
    return in_maps


def assemble_output(cfg: Cfg, results, hidden_f32):
    """results: list of per-core dicts with out/vout/gout.
    hidden_f32: [NTOK, HC, HID]. Returns [NTOK, HC, HID] f32."""
    n, T, HC, HID, NTB = cfg.n_cores, cfg.T_own, cfg.HC, cfg.HID, cfg.NTB
    outs = []
    for c in range(n):
        sil = np.asarray(results[c]["out"], np.float32)      # [C, T]
        sil = sil.reshape(HC, HID, T).transpose(2, 0, 1)     # [T, HC, HID]
        val = np.asarray(results[c]["vout"], np.float32).T   # [T, HID]
        g = np.asarray(results[c]["gout"], np.float32)       # [128, HC*NTB]
        g = g.reshape(128, HC, NTB)[:, :, 1:]                # drop halo block
        gate = g.transpose(2, 0, 1).reshape(T, HC)           # [T, HC]
        out = sil + gate[:, :, None] * val[:, None, :]
        outs.append(out)
    full = np.concatenate(outs, axis=0)
    return full + hidden_f32


_CACHED = {}


def _get_program(key, cfg):
    if key not in _CACHED:
        _CACHED[key] = build_program(cfg)
    return _CACHED[key]


def prepare(inputs, cfg=None):
    hash_indices = np.asarray(inputs["hash_indices"])
    hidden = np.asarray(inputs["hidden_states"], dtype=np.float32)
    B, S, NH = hash_indices.shape
    NTOK = B * S
    if cfg is None:
        cfg = Cfg(n_cores=8, T_own=NTOK // 8)
    nc = _get_program(("hw", cfg.T_own, cfg.VPAD, cfg.use_silu,
                       cfg.ght_pool, cfg.loop_n, cfg.pack,
                       cfg.skip_gather, cfg.phase_limit), cfg)
    hidx = np.ascontiguousarray(
        hash_indices.reshape(NTOK, NH).astype(np.int32))
    hid_flat = np.ascontiguousarray(hidden.reshape(NTOK, cfg.C))
    in_maps = make_in_maps(
        cfg, hidx, hid_flat,
        np.asarray(inputs["emb_table"], np.float32),
        np.asarray(inputs["offsets"]),
        np.asarray(inputs["w_v"], np.float32),
        np.asarray(inputs["w_k"], np.float32),
        np.asarray(inputs["norm_h_w"], np.float32),
        np.asarray(inputs["norm_k_w"], np.float32),
        np.asarray(inputs["sc_norm_w"], np.float32),
        np.asarray(inputs["conv_w"], np.float32),
        seq_len=S)
    return cfg, nc, in_maps


def kernel(**inputs):
    hidden = np.asarray(inputs["hidden_states"], dtype=np.float32)
    B, S, NH = np.asarray(inputs["hash_indices"]).shape
    NTOK = B * S
    cfg, nc, in_maps = prepare(inputs)
    res = run_bass_kernel_spmd(nc, in_maps, core_ids=list(range(8)))
    full = assemble_output(cfg, res.results,
                           hidden.reshape(NTOK, cfg.HC, cfg.HID))
    return full.reshape(B, S, cfg.HC, cfg.HID).astype(np.float32)
